# revision 1
# baseline (speedup 1.0000x reference)
"""BinaryDense kernel for Trainium2: out = sign(x) @ sign(w).

Full shapes: x [8192, 4096] f32, w [4096, 4096] f32 -> out [8192, 4096] f32.

Sharding over 8 NeuronCores (2D): x rows split 4 ways, w columns split 2 ways.
Each core computes a [2048, 2048] output block from x_shard [2048, 4096] and
w_shard [4096, 2048]. The host slices inputs and reassembles the output; no
collectives are needed.

Per-core kernel: binarize both operands on-chip to fp8e4 (+-1 is exact,
products are +-1 and sums are integers <= 4096, so fp32 PSUM accumulation is
exact), keep binarized w resident in SBUF, and run fp8 DoubleRow matmuls
(2 contraction tiles per pass).

Input handling:
  - Loads are SWDGE casting DMAs (f32 DRAM -> bf16 SBUF).  bf16 rounding
    cannot flip a sign (values below the smallest bf16 subnormal would need
    |x| < 5e-41; probability ~0 for randn inputs), so sign() is unaffected.
  - x tiles are PE-transposed directly out of the bf16 staging buffer; the
    sign binarization is FUSED into the PSUM->SBUF eviction (ACT Sign
    activation, bf16 psum -> fp8 SBUF).  No separate sign pass over x.
  - w signs alternate between ACT (1-op Sign activation) and DVE (2-op
    min/max-clamp sign) so no single engine paces the w pipeline.

Scheduling: w is loaded in two COLUMN-HALF passes.  DMA order is x0, the
first w half (interleaving x1), the remaining x blocks, then the second w
half — whose signs land during the dense phase, where ACT/DVE have slack.
Compute runs as two sweeps over the row blocks (one per w column-half):
sweep 1 starts as soon as the first half of w is signed (~35us in) and
carries all transposes (emitted TLOOK blocks ahead); each (row block,
column half) is a PE burst of 2x16 DoubleRow matmuls accumulating full K
into one psum bank per 512-wide chunk, evicted with one copy and written
out per half-row so staging tiles recycle immediately.  m-block 0's sweep-1
matmuls are emitted pair-major so they consume w tiles as they arrive.
"""

import numpy as np

import concourse.mybir as mybir
import concourse.tile as tile
from concourse import bacc
from concourse.bass_utils import run_bass_kernel_spmd
from concourse.masks import make_identity

P = 128
N_CORES = 8
RM, RN = 4, 2            # row shards of x, column shards of w
M_FULL, K, N_FULL = 8192, 4096, 4096
M_SH, N_SH = M_FULL // RM, N_FULL // RN   # 2048, 2048
KB = K // P              # 32 contraction tiles
MB = M_SH // P           # 16 row blocks
NB = N_SH // 512         # 4 psum-width column chunks
TLOOK = 2                # transpose lookahead (m-blocks) over matmuls
X_HEAD = 2               # x blocks loaded before w
TGRP = 8                 # transposes sharing one psum tile (bf16: 1 bank)

USE_FP8_DR = True

F32 = mybir.dt.float32
BF16 = mybir.dt.bfloat16
FP8 = mybir.dt.float8e4
I16 = mybir.dt.int16

_NC_CACHE = None


def build_nc():
    mm_dt = FP8 if USE_FP8_DR else BF16

    nc = bacc.Bacc("TRN2", target_bir_lowering=False, debug=False,
                   num_devices=N_CORES)
    x = nc.dram_tensor("x", [M_SH, K], F32, kind="ExternalInput").ap()
    w = nc.dram_tensor("w", [K, N_SH], F32, kind="ExternalInput").ap()
    out = nc.dram_tensor("out", [M_SH, N_SH], I16, kind="ExternalOutput").ap()

    with tile.TileContext(nc) as tc:
        with (
            tc.tile_pool(name="const", bufs=1) as const_pool,
            tc.tile_pool(name="wbin", bufs=1) as wbin_pool,
            tc.tile_pool(name="xTr", bufs=1) as xT_pool,
            tc.tile_pool(name="ftmp", bufs=4) as ftmp_pool,
            tc.tile_pool(name="obuf", bufs=3) as obuf_pool,
            tc.tile_pool(name="psumT", bufs=3, space="PSUM") as psumT_pool,
            tc.tile_pool(name="psumO", bufs=5, space="PSUM") as psumO_pool,
        ):
            ident = const_pool.tile([P, P], BF16)
            make_identity(nc, ident)

            # Binarized, resident operands: w as [p, kb, n]; xT as [p, mb, kb, m]
            wbin = wbin_pool.tile([P, KB, N_SH], mm_dt)
            xT = xT_pool.tile([P, MB, KB, P], mm_dt)
            xstage = [None] * MB

            def load_x(mb):
                xt = ftmp_pool.tile([P, K], BF16, tag="xstage")
                nc.gpsimd.dma_start(out=xt[:], in_=x[mb * P:(mb + 1) * P, :])
                xstage[mb] = xt

            w3d = w.rearrange("(o p) n -> p o n", p=P)   # [128, KB, N_SH]
            NH = N_SH // 2

            def load_w(kb2, half):
                # Load one column-half of two k-tiles per DMA (0.5 MiB dest).
                nsl = slice(half * NH, (half + 1) * NH)
                wt = ftmp_pool.tile([P, 2, NH], BF16, tag="wstage")
                nc.gpsimd.dma_start(
                    out=wt[:], in_=w3d[:, 2 * kb2:2 * kb2 + 2, nsl])
                dst = wbin[:, 2 * kb2:2 * kb2 + 2, nsl]
                # w signs split between ACT (1-op Sign) and DVE (2-op clamp
                # sign: min(max(x*HUGE,-1),1), exact except |x| below the
                # smallest bf16 subnormal, probability ~0 for randn inputs;
                # sign(0)=0 is preserved).  The second half leans on DVE,
                # since ACT is saturated by transpose evictions during the
                # dense phase where those signs land.
                on_act = (kb2 % 8 < 5) if half == 0 else (kb2 % 4 == 0)
                if on_act:
                    nc.scalar.sign(dst, wt[:])
                else:
                    nc.vector.tensor_scalar(
                        dst, wt[:], 3.4e38, -1.0,
                        mybir.AluOpType.mult, mybir.AluOpType.max)
                    nc.vector.tensor_scalar(
                        dst, dst, 1.0, None, mybir.AluOpType.min)

            def transposes(mb, early=False):
                xt = xstage[mb]
                for g in range(KB // TGRP):
                    pt = psumT_pool.tile([P, TGRP, P], BF16, tag="psumT")
                    for j in range(TGRP):
                        kb = g * TGRP + j
                        nc.tensor.transpose(
                            pt[:, j, :], xt[:, kb * P:(kb + 1) * P], ident[:])
                    # Fused sign + downconvert during PSUM eviction.  The
                    # prologue blocks evict via the DVE clamp sign instead,
                    # since ACT is saturated by the first w-half signs then.
                    dst = xT[:, mb, g * TGRP:(g + 1) * TGRP, :]
                    if early:
                        nc.vector.tensor_scalar(
                            dst, pt[:], 3.4e38, -1.0,
                            mybir.AluOpType.mult, mybir.AluOpType.max)
                        nc.vector.tensor_scalar(
                            dst, dst, 1.0, None, mybir.AluOpType.min)
                    else:
                        nc.scalar.sign(dst, pt[:])

            def mm(po, mb, kb, nsl, start, stop):
                if USE_FP8_DR:
                    nc.tensor.matmul(
                        po[:], xT[:, mb, kb:kb + 2, :], wbin[:, kb:kb + 2, nsl],
                        start=start, stop=stop,
                        perf_mode=mybir.MatmulPerfMode.DoubleRow)
                else:
                    nc.tensor.matmul(
                        po[:], xT[:, mb, kb, :], wbin[:, kb, nsl],
                        start=start, stop=False)
                    nc.tensor.matmul(
                        po[:], xT[:, mb, kb + 1, :], wbin[:, kb + 1, nsl],
                        start=False, stop=stop)

            npair = KB // 2
            nsls = [slice(nb * 512, (nb + 1) * 512) for nb in range(NB)]

            def bass_ts(j):
                return slice(j * 512, (j + 1) * 512)

            def matmuls(mb, half, surf=False):
                # One column-half of one row block: 2 psum groups, one int16
                # staging tile (values are integers <= 4096, exactly
                # representable; the host widens back to f32), one out DMA.
                nbs = [2 * half, 2 * half + 1]
                ob = obuf_pool.tile([P, N_SH // 2], I16, tag="obuf")
                pos = [psumO_pool.tile([P, 512], F32, tag="psumO", name="po")
                       for _ in nbs]
                if surf:
                    # Pair-major emission: each arriving w pair immediately
                    # feeds the matmuls, so this block's matmuls overlap the
                    # w load instead of waiting for it.
                    for i in range(npair):
                        for j, nb in enumerate(nbs):
                            mm(pos[j], mb, 2 * i, nsls[nb],
                               start=(i == 0), stop=(i == npair - 1))
                else:
                    for j, nb in enumerate(nbs):
                        for i in range(npair):
                            mm(pos[j], mb, 2 * i, nsls[nb],
                               start=(i == 0), stop=(i == npair - 1))
                for j, nb in enumerate(nbs):
                    nc.vector.tensor_copy(
                        out=ob[:, bass_ts(j)], in_=pos[j][:])
                nc.sync.dma_start(
                    out=out[mb * P:(mb + 1) * P,
                            half * (N_SH // 2):(half + 1) * (N_SH // 2)],
                    in_=ob[:])

            # DMA issue order: a couple of x blocks, the first column-half
            # of w, the remaining x blocks, then the second half of w.  Its
            # signs land in the dense phase, where ACT/DVE have slack.
            load_x(0)
            for kb2 in range(KB // 2):
                load_w(kb2, 0)
                if kb2 == 3:
                    load_x(1)
            for mb in range(X_HEAD, MB):
                load_x(mb)
            for kb2 in range(KB // 2):
                load_w(kb2, 1)

            # Compute in two sweeps over the row blocks, one per w
            # column-half; sweep 1 starts as soon as the first half of w is
            # signed, sweep 2 as the second half lands behind it.
            for mb in range(TLOOK):
                transposes(mb)
            for mb in range(MB):
                matmuls(mb, 0, surf=(mb == 0))
                if mb + TLOOK < MB:
                    transposes(mb + TLOOK)
            for mb in range(MB):
                matmuls(mb, 1)

    nc.compile()
    return nc


def get_nc():
    global _NC_CACHE
    if _NC_CACHE is None:
        _NC_CACHE = build_nc()
    return _NC_CACHE


def kernel(x: np.ndarray, w: np.ndarray) -> np.ndarray:
    x = np.asarray(x, dtype=np.float32)
    w = np.asarray(w, dtype=np.float32)
    assert x.shape == (M_FULL, K) and w.shape == (K, N_FULL)

    nc = get_nc()
    in_maps = []
    for c in range(N_CORES):
        mi, ni = divmod(c, RN)
        in_maps.append({
            "x": np.ascontiguousarray(x[mi * M_SH:(mi + 1) * M_SH, :]),
            "w": np.ascontiguousarray(w[:, ni * N_SH:(ni + 1) * N_SH]),
        })
    res = run_bass_kernel_spmd(nc, in_maps, list(range(N_CORES)))

    out = np.empty((M_FULL, N_FULL), dtype=np.float32)
    for c in range(N_CORES):
        mi, ni = divmod(c, RN)
        out[mi * M_SH:(mi + 1) * M_SH, ni * N_SH:(ni + 1) * N_SH] = \
            res.results[c]["out"].astype(np.float32)
    return out



# revision 3
# speedup vs baseline: 1.2417x; 1.2417x over previous
"""BinaryDense kernel for Trainium2: out = sign(x) @ sign(w).

Full shapes: x [8192, 4096] f32, w [4096, 4096] f32 -> out [8192, 4096] f32.
Sharding over 8 NeuronCores: x rows split 4 ways, w columns split 2 ways;
each core computes a [2048, 2048] block.  No collectives.

Core ideas (cost-model-driven):
  - fp8e5 cast-loads (SWDGE): IEEE casts preserve the sign BIT (even on
    underflow to +-0) and only the sign bit matters -> input DMA halves.
  - Bitwise sign on uint16 views: (r & 0x8080) | 0x3C3C == +-1.0 fp8e5 in
    both packed bytes.  Single DVE op per 2 elements, exact.
  - Matmuls: fp8 DoubleRowSwInterleave (0.5 cycles/row).  The k-pair-packed
    transposed x IS the interleaved stationary operand; w cast-loads land
    directly in the plane-separated moving layout [p, j, t, n].  The mode
    reads stationary columns in reverse order, so the host pre-reverses x
    rows within each 128-row block.
  - PSUM f32 accumulation is exact (+-1 products); int16 out; host widens.

Schedule: the finish time is ~(w-stream end + 97us), so the w stream runs
UNBROKEN on the Pool/SWDGE queue right after the first x chunk.  The first
two x chunks are transposed on the PE (raw fp8 pairs as u16; the sign is
fused into the DVE psum->SBUF eviction), which costs PE time only where PE
is delivery-paced anyway and keeps cross-queue DMA hops out of the w
stream.  Remaining chunks use XBAR DMA-transposes after the stream, where
the DMA device has slack.  Steady-state matmuls run nb-major so psum banks
recycle incrementally (6 matmul banks + 2 transpose banks).

Queue map: Pool=cast loads | DVE=signs+evictions | SP=XBAR transposes |
ACT=output DMA issue | PE=matmuls + first-two-chunk transposes.
"""

import numpy as np

import concourse.mybir as mybir
import concourse.tile as tile
from concourse import bacc
from concourse.bass_utils import run_bass_kernel_spmd
from concourse.masks import make_identity

P = 128
N_CORES = 8
RM, RN = 4, 2
M_FULL, K, N_FULL = 8192, 4096, 4096
M_SH, N_SH = M_FULL // RM, N_FULL // RN   # 2048, 2048
MB = M_SH // P           # 16 m-blocks
JB = K // 256            # 16 k-groups (DoubleRow: 2 planes x 128)
NB = N_SH // 512         # 4 psum-width chunks
XC = MB // 2             # 8 x-chunks of 2 m-blocks
TGRP = 8                 # u16 128-blocks per PE-transpose psum group

F32 = mybir.dt.float32
FP8 = mybir.dt.float8e5
U16 = mybir.dt.uint16
I16 = mybir.dt.int16

AND_MASK = 0x8080
OR_MASK = 0x3C3C
DRSW = mybir.MatmulPerfMode.DoubleRowSwInterleave

_NC_CACHE = None

# DMA schedule pins in "ms" for tc.tile_wait_until (1e6 ns units)
PINS = {
    "xc1": 0.0264, "c2": 0.0294, "c3": 0.0324, "c4": 0.0354,
    "c5": 0.0423, "c6": 0.0488, "c7": 0.0517,
    "T2": 0.0387, "T3": 0.0452, "T4": 0.0546, "T5": 0.0614,
    "T6": 0.0674, "T7": 0.0720,
}


def build_nc():
    nc = bacc.Bacc("TRN2", target_bir_lowering=False, debug=False,
                   num_devices=N_CORES)
    x = nc.dram_tensor("x", [M_SH, K], F32, kind="ExternalInput").ap()
    w = nc.dram_tensor("w", [K, N_SH], F32, kind="ExternalInput").ap()
    out = nc.dram_tensor("out", [M_SH, N_SH], I16, kind="ExternalOutput").ap()

    with tile.TileContext(nc) as tc:
        with (
            tc.tile_pool(name="const", bufs=1) as const_pool,
            tc.tile_pool(name="xT", bufs=1) as xT_pool,
            tc.tile_pool(name="wbin", bufs=1) as w_pool,
            tc.tile_pool(name="xs", bufs=5) as xs_pool,
            tc.tile_pool(name="ws", bufs=4) as ws_pool,
            tc.tile_pool(name="obuf", bufs=3) as ob_pool,
            tc.tile_pool(name="psum", bufs=6, space="PSUM") as psum_pool,
            tc.tile_pool(name="psumT", bufs=2, space="PSUM") as psumT_pool,
        ):
            ident = const_pool.tile([P, P], mybir.dt.int16)

            # xT u16[p, mb, j, m] = fp8 pair (k=256j+2p, +1) of row m
            xT = xT_pool.tile([P, MB, JB, P], U16)
            # wsgn[p, j, t, n] = sign(w[256j+2p+t, n])
            wsgn = w_pool.tile([P, JB, 2, N_SH], FP8)
            w4d = w.rearrange("(j p t) n -> p j t n", p=P, t=2)
            # x chunk c covers m-blocks 2c, 2c+1: partition p holds rows
            # 256c+p and 256c+128+p
            x3d = x.rearrange("(c two p) k -> p c two k", two=2, p=P)

            xstage = [None] * XC

            def sign_u16(dst, src):
                nc.vector.tensor_scalar(
                    dst, src, AND_MASK, OR_MASK,
                    mybir.AluOpType.bitwise_and, mybir.AluOpType.bitwise_or)

            def load_w(j):
                wr = ws_pool.tile([P, 2, N_SH], FP8, tag="wr")
                nc.gpsimd.dma_start(out=wr[:], in_=w4d[:, j])
                sign_u16(wsgn[:, j, :, :].bitcast(U16), wr[:].bitcast(U16))

            def load_x_raw(c):
                # raw staging for PE-transposed chunks (sign happens at the
                # psum eviction)
                xs = xs_pool.tile([P, 2, K], FP8, tag="xs")
                nc.gpsimd.dma_start(out=xs[:], in_=x3d[:, c])
                xstage[c] = xs

            def load_x_raw_half(c, half):
                if half == 0:
                    xstage[c] = xs_pool.tile([P, 2, K], FP8, tag="xs", name="xsh")
                nc.gpsimd.dma_start(
                    out=xstage[c][:, half, :], in_=x3d[:, c, half, :])

            def sign_x(c):
                # in-place sign of a raw-staged chunk (DVE); emitted in need
                # order so it can never block a due psum eviction
                sign_u16(xstage[c][:].bitcast(U16), xstage[c][:].bitcast(U16))

            def pe_transpose_half(c, half):
                # One m-block (16 u16-blocks) of chunk c through the PE in
                # two TGRP groups; sign is fused into the DVE eviction.
                # The PE transpose runs on BF16 *views* of the u16 pair
                # data: transpose mode is pure routing and bit-preserving
                # (verified on HW for all 65536 patterns), and bf16 is a
                # compiler-accepted PE dtype while u16 is not.  The psum
                # tiles are F32-shaped so the same pool doubles as warmup
                # accumulator space.
                xu = xstage[c][:].bitcast(mybir.dt.bfloat16)   # [P, 2, 2048]
                mbi = 2 * c + half
                for g in range(2):
                    pt = psumT_pool.tile([P, 512], F32, tag="pt", name="pt")
                    ptb = pt[:].bitcast(mybir.dt.bfloat16)     # [P, 1024]
                    for i in range(TGRP):
                        b = TGRP * g + i
                        nc.tensor.transpose(
                            ptb[:, i * P:(i + 1) * P],
                            xu[:, half, b * P:(b + 1) * P],
                            ident[:].bitcast(mybir.dt.bfloat16))
                    sign_u16(xT[:, mbi, TGRP * g:TGRP * (g + 1), :],
                             pt[:].bitcast(U16).rearrange(
                                 "p (a b) -> p a b", a=TGRP))

            def transpose_x(c):
                nc.sync.dma_start(
                    out=xT[:, 2 * c:2 * c + 2, :, :],
                    in_=xstage[c][:].bitcast(U16), transpose=True)

            def mm(po, mb, j, nb, start, stop):
                nc.tensor.matmul(
                    po[:], xT[:, mb, j, :].bitcast(FP8),
                    wsgn[:, j, :, nb * 512:(nb + 1) * 512],
                    start=start, stop=stop, perf_mode=DRSW)

            # ---- prologue loads: xc0, then the unbroken w stream, xc1 ----
            # Input DMAs after the w stream are pinned (tile_wait_until) to a
            # hand-planned timeline so the scheduler's enforced DMA order
            # matches what is actually achievable at runtime.
            load_w(0)
            load_w(1)
            load_x_raw(0)
            # identity built after the first loads so its gpsimd memset does
            # not delay the w stream's first SWDGE preps
            make_identity(nc, ident)
            for j in range(2, JB):
                load_w(j)
            with tc.tile_wait_until(PINS['xc1']):
                load_x_raw(1)

            # c0 PE transposes: gated only on the xc0 load; run pre-warmup
            pe_transpose_half(0, 0)
            pe_transpose_half(0, 1)

            # ---- warmup: mb0+mb1 j-interleaved, 6 matmul banks:
            # mb0 all 4 chunks, mb1 chunks 0-1; mb1 chunks 2-3 follow
            # nb-major right after.
            po6 = [psum_pool.tile([P, 512], F32, tag="po", name=f"po{i}")
                   for i in range(6)]
            for j in range(JB):
                for nb in range(NB):
                    mm(po6[nb], 0, j, nb, start=(j == 0), stop=(j == JB - 1))
                for nb in range(2):
                    mm(po6[4 + nb], 1, j, nb,
                       start=(j == 0), stop=(j == JB - 1))

            ob0 = ob_pool.tile([P, N_SH], I16, tag="ob")
            for nb in range(NB):
                nc.vector.tensor_copy(
                    out=ob0[:, nb * 512:(nb + 1) * 512], in_=po6[nb][:])
            nc.scalar.dma_start(out=out[0:P, :], in_=ob0[:])

            ob1 = ob_pool.tile([P, N_SH], I16, tag="ob")
            for nb in range(2):
                nc.vector.tensor_copy(
                    out=ob1[:, nb * 512:(nb + 1) * 512], in_=po6[4 + nb][:])
            # mb1 chunks 2-3 (nb-major) while c1 transposes interleave
            for nb in range(2, NB):
                po = psum_pool.tile([P, 512], F32, tag="po", name="po")
                for j in range(JB):
                    mm(po, 1, j, nb, start=(j == 0), stop=(j == JB - 1))
                if nb == 2:
                    pe_transpose_half(1, 0)
                nc.vector.tensor_copy(
                    out=ob1[:, nb * 512:(nb + 1) * 512], in_=po[:])
            nc.scalar.dma_start(out=out[P:2 * P, :], in_=ob1[:])

            # XBAR-transposed chunks: loads stream right after w on Pool;
            # signs/transposes are emitted in need order below
            for c, ms in ((2, PINS['c2']), (3, PINS['c3']), (4, PINS['c4']),
                          (5, PINS['c5']), (6, PINS['c6'])):
                with tc.tile_wait_until(ms):
                    load_x_raw(c)
            sign_x(2)
            with tc.tile_wait_until(PINS['T2']):
                transpose_x(2)
            sign_x(3)
            with tc.tile_wait_until(PINS['T3']):
                transpose_x(3)

            # ---- steady: mb2..15, nb-major, incremental bank recycling ----
            T_PINS = {4: PINS['T4'], 5: PINS['T5'], 6: PINS['T6'],
                      7: PINS['T7']}
            for mb in range(2, MB):
                if mb == 2:
                    pe_transpose_half(1, 1)
                if 2 <= mb <= 4:
                    sign_x(mb + 2)
                    with tc.tile_wait_until(T_PINS[mb + 2]):
                        transpose_x(mb + 2)
                elif mb == 5:
                    with tc.tile_wait_until(PINS['c7']):
                        load_x_raw(7)
                    sign_x(7)
                    with tc.tile_wait_until(T_PINS[7]):
                        transpose_x(7)
                last = (mb == MB - 1)
                ob = ob_pool.tile([P, N_SH], I16, tag="ob")
                for nb in range(NB):
                    po = psum_pool.tile([P, 512], F32, tag="po", name="po")
                    for j in range(JB):
                        mm(po, mb, j, nb, start=(j == 0), stop=(j == JB - 1))
                    nsl = slice(nb * 512, (nb + 1) * 512)
                    if not last:
                        nc.vector.tensor_copy(out=ob[:, nsl], in_=po[:])
                    else:
                        nc.vector.tensor_copy(out=ob[:, nsl], in_=po[:])
                        nc.scalar.dma_start(
                            out=out[mb * P:(mb + 1) * P, nsl], in_=ob[:, nsl])
                if not last:
                    nc.scalar.dma_start(
                        out=out[mb * P:(mb + 1) * P, :], in_=ob[:])

    nc.compile()
    return nc


def get_nc():
    global _NC_CACHE
    if _NC_CACHE is None:
        _NC_CACHE = build_nc()
    return _NC_CACHE


def kernel(x: np.ndarray, w: np.ndarray) -> np.ndarray:
    x = np.asarray(x, dtype=np.float32)
    w = np.asarray(w, dtype=np.float32)
    assert x.shape == (M_FULL, K) and w.shape == (K, N_FULL)

    nc = get_nc()
    in_maps = []
    for c in range(N_CORES):
        mi, ni = divmod(c, RN)
        # SwInterleave reads stationary columns in reverse order: pre-reverse
        # x rows within each 128-row block so output rows land in order.
        xs = x[mi * M_SH:(mi + 1) * M_SH, :]
        xs = xs.reshape(MB, P, K)[:, ::-1, :].reshape(M_SH, K)
        in_maps.append({
            "x": np.ascontiguousarray(xs),
            "w": np.ascontiguousarray(w[:, ni * N_SH:(ni + 1) * N_SH]),
        })
    res = run_bass_kernel_spmd(nc, in_maps, list(range(N_CORES)))

    out = np.empty((M_FULL, N_FULL), dtype=np.float32)
    for c in range(N_CORES):
        mi, ni = divmod(c, RN)
        out[mi * M_SH:(mi + 1) * M_SH, ni * N_SH:(ni + 1) * N_SH] = \
            res.results[c]["out"].astype(np.float32)
    return out


# revision 4
# speedup vs baseline: 1.2622x; 1.0165x over previous
"""BinaryDense kernel for Trainium2: out = sign(x) @ sign(w).

Full shapes: x [8192, 4096] f32, w [4096, 4096] f32 -> out [8192, 4096] f32.
Sharding over 8 NeuronCores: x rows split 4 ways, w columns split 2 ways;
each core computes a [2048, 2048] block.  No collectives.

Core ideas (cost-model-driven):
  - fp8e5 cast-loads (SWDGE): IEEE casts preserve the sign BIT (even on
    underflow to +-0) and only the sign bit matters -> input DMA halves.
  - Bitwise sign on uint16 views: (r & 0x8080) | 0x3C3C == +-1.0 fp8e5 in
    both packed bytes.  Single DVE op per 2 elements, exact.
  - Matmuls: fp8 DoubleRowSwInterleave (0.5 cycles/row).  The k-pair-packed
    transposed x IS the interleaved stationary operand; w cast-loads land
    directly in the plane-separated moving layout [p, j, t, n].  The mode
    reads stationary columns in reverse order, so the host pre-reverses x
    rows within each 128-row block.
  - PSUM f32 accumulation is exact (+-1 products); int16 out; host widens.

Schedule: the finish time is ~(w-stream end + 97us), so the w stream runs
UNBROKEN on the Pool/SWDGE queue right after the first x chunk.  The first
two x chunks are transposed on the PE (raw fp8 pairs as u16; the sign is
fused into the DVE psum->SBUF eviction), which costs PE time only where PE
is delivery-paced anyway and keeps cross-queue DMA hops out of the w
stream.  Remaining chunks use XBAR DMA-transposes after the stream, where
the DMA device has slack.  Steady-state matmuls run nb-major so psum banks
recycle incrementally (6 matmul banks + 2 transpose banks).

Queue map: Pool=cast loads | DVE=signs+evictions | SP=XBAR transposes |
ACT=output DMA issue | PE=matmuls + first-two-chunk transposes.
"""

import numpy as np

import concourse.mybir as mybir
import concourse.tile as tile
from concourse import bacc
from concourse.bass_utils import run_bass_kernel_spmd
from concourse.masks import make_identity

P = 128
N_CORES = 8
RM, RN = 4, 2
M_FULL, K, N_FULL = 8192, 4096, 4096
M_SH, N_SH = M_FULL // RM, N_FULL // RN   # 2048, 2048
MB = M_SH // P           # 16 m-blocks
JB = K // 256            # 16 k-groups (DoubleRow: 2 planes x 128)
NB = N_SH // 512         # 4 psum-width chunks
XC = MB // 2             # 8 x-chunks of 2 m-blocks
TGRP = 8                 # u16 128-blocks per PE-transpose psum group

F32 = mybir.dt.float32
FP8 = mybir.dt.float8e5
U16 = mybir.dt.uint16
I16 = mybir.dt.int16

AND_MASK = 0x8080
OR_MASK = 0x3C3C
DRSW = mybir.MatmulPerfMode.DoubleRowSwInterleave

_NC_CACHE = None

# DMA schedule pins in "ms" for tc.tile_wait_until (1e6 ns units)
PINS = {
    "xc1": 0.0264, "c2": 0.0294, "c3": 0.0324, "c4": 0.0354,
    "c5": 0.0423, "c6": 0.0488, "c7": 0.0517,
    "T2": 0.0387, "T3": 0.0452, "T4": 0.0546, "T5": 0.0614,
    "T6": 0.0674, "T7": 0.0720,
}


def build_nc():
    nc = bacc.Bacc("TRN2", target_bir_lowering=False, debug=False,
                   num_devices=N_CORES)
    x = nc.dram_tensor("x", [M_SH, K], F32, kind="ExternalInput").ap()
    w = nc.dram_tensor("w", [K, N_SH], F32, kind="ExternalInput").ap()
    out = nc.dram_tensor("out", [M_SH, N_SH], I16, kind="ExternalOutput").ap()

    with tile.TileContext(nc) as tc:
        with (
            tc.tile_pool(name="const", bufs=1) as const_pool,
            tc.tile_pool(name="xT", bufs=1) as xT_pool,
            tc.tile_pool(name="wbin", bufs=1) as w_pool,
            tc.tile_pool(name="xs", bufs=5) as xs_pool,
            tc.tile_pool(name="ws", bufs=5) as ws_pool,
            tc.tile_pool(name="obuf", bufs=4) as ob_pool,
            tc.tile_pool(name="psum", bufs=6, space="PSUM") as psum_pool,
            tc.tile_pool(name="psumT", bufs=2, space="PSUM") as psumT_pool,
        ):
            ident = const_pool.tile([P, P], mybir.dt.int16)

            # xT u16[p, mb, j, m] = fp8 pair (k=256j+2p, +1) of row m
            xT = xT_pool.tile([P, MB, JB, P], U16)
            # wsgn[p, j, t, n] = sign(w[256j+2p+t, n])
            wsgn = w_pool.tile([P, JB, 2, N_SH], FP8)
            w4d = w.rearrange("(j p t) n -> p j t n", p=P, t=2)
            # x chunk c covers m-blocks 2c, 2c+1: partition p holds rows
            # 256c+p and 256c+128+p
            x3d = x.rearrange("(c two p) k -> p c two k", two=2, p=P)

            xstage = [None] * XC

            def sign_u16(dst, src):
                nc.vector.tensor_scalar(
                    dst, src, AND_MASK, OR_MASK,
                    mybir.AluOpType.bitwise_and, mybir.AluOpType.bitwise_or)

            def load_w(j):
                wr = ws_pool.tile([P, 2, N_SH], FP8, tag="wr")
                nc.gpsimd.dma_start(out=wr[:], in_=w4d[:, j])
                sign_u16(wsgn[:, j, :, :].bitcast(U16), wr[:].bitcast(U16))

            def load_x_raw(c):
                # raw staging for PE-transposed chunks (sign happens at the
                # psum eviction)
                xs = xs_pool.tile([P, 2, K], FP8, tag="xs")
                nc.gpsimd.dma_start(out=xs[:], in_=x3d[:, c])
                xstage[c] = xs

            def load_x_raw_half(c, half):
                if half == 0:
                    xstage[c] = xs_pool.tile([P, 2, K], FP8, tag="xs", name="xsh")
                nc.gpsimd.dma_start(
                    out=xstage[c][:, half, :], in_=x3d[:, c, half, :])

            def sign_x(c):
                # in-place sign of a raw-staged chunk (DVE); emitted in need
                # order so it can never block a due psum eviction
                sign_u16(xstage[c][:].bitcast(U16), xstage[c][:].bitcast(U16))

            def pe_transpose_half(c, half):
                # One m-block (16 u16-blocks) of chunk c through the PE in
                # two TGRP groups; sign is fused into the DVE eviction.
                # The PE transpose runs on BF16 *views* of the u16 pair
                # data: transpose mode is pure routing and bit-preserving
                # (verified on HW for all 65536 patterns), and bf16 is a
                # compiler-accepted PE dtype while u16 is not.  The psum
                # tiles are F32-shaped so the same pool doubles as warmup
                # accumulator space.
                xu = xstage[c][:].bitcast(mybir.dt.bfloat16)   # [P, 2, 2048]
                mbi = 2 * c + half
                for g in range(2):
                    pt = psumT_pool.tile([P, 512], F32, tag="pt", name="pt")
                    ptb = pt[:].bitcast(mybir.dt.bfloat16)     # [P, 1024]
                    for i in range(TGRP):
                        b = TGRP * g + i
                        nc.tensor.transpose(
                            ptb[:, i * P:(i + 1) * P],
                            xu[:, half, b * P:(b + 1) * P],
                            ident[:].bitcast(mybir.dt.bfloat16))
                    sign_u16(xT[:, mbi, TGRP * g:TGRP * (g + 1), :],
                             pt[:].bitcast(U16).rearrange(
                                 "p (a b) -> p a b", a=TGRP))

            def transpose_x(c):
                nc.sync.dma_start(
                    out=xT[:, 2 * c:2 * c + 2, :, :],
                    in_=xstage[c][:].bitcast(U16), transpose=True)

            def mm(po, mb, j, nb, start, stop):
                nc.tensor.matmul(
                    po[:], xT[:, mb, j, :].bitcast(FP8),
                    wsgn[:, j, :, nb * 512:(nb + 1) * 512],
                    start=start, stop=stop, perf_mode=DRSW)

            # ---- prologue loads: xc0, then the unbroken w stream, xc1 ----
            # Input DMAs after the w stream are pinned (tile_wait_until) to a
            # hand-planned timeline so the scheduler's enforced DMA order
            # matches what is actually achievable at runtime.
            load_w(0)
            load_w(1)
            load_x_raw(0)
            # identity built after the first loads so its gpsimd memset does
            # not delay the w stream's first SWDGE preps
            make_identity(nc, ident)
            for j in range(2, JB):
                load_w(j)
            with tc.tile_wait_until(PINS['xc1']):
                load_x_raw(1)

            # c0 PE transposes: gated only on the xc0 load; run pre-warmup
            pe_transpose_half(0, 0)
            pe_transpose_half(0, 1)

            # ---- warmup: mb0+mb1 j-interleaved, 6 matmul banks:
            # mb0 all 4 chunks, mb1 chunks 0-1; mb1 chunks 2-3 follow
            # nb-major right after.
            po6 = [psum_pool.tile([P, 512], F32, tag="po", name=f"po{i}")
                   for i in range(6)]
            for j in range(JB):
                for nb in range(NB):
                    mm(po6[nb], 0, j, nb, start=(j == 0), stop=(j == JB - 1))
                for nb in range(2):
                    mm(po6[4 + nb], 1, j, nb,
                       start=(j == 0), stop=(j == JB - 1))

            ob0 = ob_pool.tile([P, N_SH], I16, tag="ob")
            for nb in range(NB):
                nc.vector.tensor_copy(
                    out=ob0[:, nb * 512:(nb + 1) * 512], in_=po6[nb][:])
            nc.scalar.dma_start(out=out[0:P, :], in_=ob0[:])

            ob1 = ob_pool.tile([P, N_SH], I16, tag="ob")
            for nb in range(2):
                nc.vector.tensor_copy(
                    out=ob1[:, nb * 512:(nb + 1) * 512], in_=po6[4 + nb][:])
            # mb1 chunks 2-3 (nb-major) while c1 transposes interleave
            for nb in range(2, NB):
                po = psum_pool.tile([P, 512], F32, tag="po", name="po")
                for j in range(JB):
                    mm(po, 1, j, nb, start=(j == 0), stop=(j == JB - 1))
                if nb == 2:
                    pe_transpose_half(1, 0)
                nc.vector.tensor_copy(
                    out=ob1[:, nb * 512:(nb + 1) * 512], in_=po[:])
            nc.scalar.dma_start(out=out[P:2 * P, :], in_=ob1[:])

            # XBAR-transposed chunks: loads stream right after w on Pool;
            # signs/transposes are emitted in need order below
            for c, ms in ((2, PINS['c2']), (3, PINS['c3']), (4, PINS['c4']),
                          (5, PINS['c5']), (6, PINS['c6'])):
                with tc.tile_wait_until(ms):
                    load_x_raw(c)
            sign_x(2)
            with tc.tile_wait_until(PINS['T2']):
                transpose_x(2)
            sign_x(3)
            with tc.tile_wait_until(PINS['T3']):
                transpose_x(3)

            # ---- steady: mb2..15, nb-major, incremental bank recycling ----
            T_PINS = {4: PINS['T4'], 5: PINS['T5'], 6: PINS['T6'],
                      7: PINS['T7']}
            for mb in range(2, MB):
                if mb == 2:
                    pe_transpose_half(1, 1)
                if 2 <= mb <= 4:
                    sign_x(mb + 2)
                    with tc.tile_wait_until(T_PINS[mb + 2]):
                        transpose_x(mb + 2)
                elif mb == 5:
                    with tc.tile_wait_until(PINS['c7']):
                        load_x_raw(7)
                    sign_x(7)
                    with tc.tile_wait_until(T_PINS[7]):
                        transpose_x(7)
                last = (mb == MB - 1)
                ob = ob_pool.tile([P, N_SH], I16, tag="ob")
                for nb in range(NB):
                    po = psum_pool.tile([P, 512], F32, tag="po", name="po")
                    for j in range(JB):
                        mm(po, mb, j, nb, start=(j == 0), stop=(j == JB - 1))
                    nsl = slice(nb * 512, (nb + 1) * 512)
                    if not last:
                        nc.vector.tensor_copy(out=ob[:, nsl], in_=po[:])
                    else:
                        nc.vector.tensor_copy(out=ob[:, nsl], in_=po[:])
                        nc.scalar.dma_start(
                            out=out[mb * P:(mb + 1) * P, nsl], in_=ob[:, nsl])
                if not last:
                    nc.scalar.dma_start(
                        out=out[mb * P:(mb + 1) * P, :], in_=ob[:])

    nc.compile()
    return nc


def get_nc():
    global _NC_CACHE
    if _NC_CACHE is None:
        _NC_CACHE = build_nc()
    return _NC_CACHE


def kernel(x: np.ndarray, w: np.ndarray) -> np.ndarray:
    x = np.asarray(x, dtype=np.float32)
    w = np.asarray(w, dtype=np.float32)
    assert x.shape == (M_FULL, K) and w.shape == (K, N_FULL)

    nc = get_nc()
    in_maps = []
    for c in range(N_CORES):
        mi, ni = divmod(c, RN)
        # SwInterleave reads stationary columns in reverse order: pre-reverse
        # x rows within each 128-row block so output rows land in order.
        xs = x[mi * M_SH:(mi + 1) * M_SH, :]
        xs = xs.reshape(MB, P, K)[:, ::-1, :].reshape(M_SH, K)
        in_maps.append({
            "x": np.ascontiguousarray(xs),
            "w": np.ascontiguousarray(w[:, ni * N_SH:(ni + 1) * N_SH]),
        })
    res = run_bass_kernel_spmd(nc, in_maps, list(range(N_CORES)))

    out = np.empty((M_FULL, N_FULL), dtype=np.float32)
    for c in range(N_CORES):
        mi, ni = divmod(c, RN)
        out[mi * M_SH:(mi + 1) * M_SH, ni * N_SH:(ni + 1) * N_SH] = \
            res.results[c]["out"].astype(np.float32)
    return out


# revision 5
# speedup vs baseline: 1.2636x; 1.0011x over previous
"""BinaryDense kernel for Trainium2: out = sign(x) @ sign(w).

Full shapes: x [8192, 4096] f32, w [4096, 4096] f32 -> out [8192, 4096] f32.
Sharding over 8 NeuronCores: x rows split 4 ways, w columns split 2 ways;
each core computes a [2048, 2048] block.  No collectives.

Core ideas (cost-model-driven):
  - fp8e5 cast-loads (SWDGE): IEEE casts preserve the sign BIT (even on
    underflow to +-0) and only the sign bit matters -> input DMA halves.
  - Bitwise sign on uint16 views: (r & 0x8080) | 0x3C3C == +-1.0 fp8e5 in
    both packed bytes.  Single DVE op per 2 elements, exact.
  - Matmuls: fp8 DoubleRowSwInterleave (0.5 cycles/row).  The k-pair-packed
    transposed x IS the interleaved stationary operand; w cast-loads land
    directly in the plane-separated moving layout [p, j, t, n].  The mode
    reads stationary columns in reverse order, so the host pre-reverses x
    rows within each 128-row block.
  - PSUM f32 accumulation is exact (+-1 products); int16 out; host widens.

Schedule: the finish time is ~(w-stream end + 97us), so the w stream runs
UNBROKEN on the Pool/SWDGE queue right after the first x chunk.  The first
two x chunks are transposed on the PE (raw fp8 pairs as u16; the sign is
fused into the DVE psum->SBUF eviction), which costs PE time only where PE
is delivery-paced anyway and keeps cross-queue DMA hops out of the w
stream.  Remaining chunks use XBAR DMA-transposes after the stream, where
the DMA device has slack.  Steady-state matmuls run nb-major so psum banks
recycle incrementally (6 matmul banks + 2 transpose banks).

Queue map: Pool=cast loads | DVE=signs+evictions | SP=XBAR transposes |
ACT=output DMA issue | PE=matmuls + first-two-chunk transposes.
"""

import numpy as np

import concourse.mybir as mybir
import concourse.tile as tile
from concourse import bacc
from concourse.bass_utils import run_bass_kernel_spmd
from concourse.masks import make_identity

P = 128
N_CORES = 8
RM, RN = 4, 2
M_FULL, K, N_FULL = 8192, 4096, 4096
M_SH, N_SH = M_FULL // RM, N_FULL // RN   # 2048, 2048
MB = M_SH // P           # 16 m-blocks
JB = K // 256            # 16 k-groups (DoubleRow: 2 planes x 128)
NB = N_SH // 512         # 4 psum-width chunks
XC = MB // 2             # 8 x-chunks of 2 m-blocks
TGRP = 8                 # u16 128-blocks per PE-transpose psum group

F32 = mybir.dt.float32
FP8 = mybir.dt.float8e5
U16 = mybir.dt.uint16
I16 = mybir.dt.int16

AND_MASK = 0x8080
OR_MASK = 0x3C3C
DRSW = mybir.MatmulPerfMode.DoubleRowSwInterleave

_NC_CACHE = None

# DMA schedule pins in "ms" for tc.tile_wait_until (1e6 ns units)
PINS = {
    "xc1": 0.0264, "c2": 0.0294, "c3": 0.0324, "c4": 0.0354,
    "c5": 0.0423, "c6": 0.0488, "c7": 0.0517,
    "T2": 0.0387, "T3": 0.0452, "T4": 0.0546, "T5": 0.0614,
    "T6": 0.0674, "T7": 0.0720,
}


def build_nc():
    nc = bacc.Bacc("TRN2", target_bir_lowering=False, debug=False,
                   num_devices=N_CORES)
    x = nc.dram_tensor("x", [M_SH, K], F32, kind="ExternalInput").ap()
    w = nc.dram_tensor("w", [K, N_SH], F32, kind="ExternalInput").ap()
    out = nc.dram_tensor("out", [M_SH, N_SH], I16, kind="ExternalOutput").ap()

    with tile.TileContext(nc) as tc:
        with (
            tc.tile_pool(name="const", bufs=1) as const_pool,
            tc.tile_pool(name="xT", bufs=1) as xT_pool,
            tc.tile_pool(name="wbin", bufs=1) as w_pool,
            tc.tile_pool(name="xs", bufs=5) as xs_pool,
            tc.tile_pool(name="ws", bufs=5) as ws_pool,
            tc.tile_pool(name="obuf", bufs=4) as ob_pool,
            tc.tile_pool(name="psum", bufs=6, space="PSUM") as psum_pool,
            tc.tile_pool(name="psumT", bufs=2, space="PSUM") as psumT_pool,
        ):
            ident = const_pool.tile([P, P], mybir.dt.int16)

            # xT u16[p, mb, j, m] = fp8 pair (k=256j+2p, +1) of row m
            xT = xT_pool.tile([P, MB, JB, P], U16)
            # wsgn[p, j, t, n] = sign(w[256j+2p+t, n])
            wsgn = w_pool.tile([P, JB, 2, N_SH], FP8)
            w4d = w.rearrange("(j p t) n -> p j t n", p=P, t=2)
            # x chunk c covers m-blocks 2c, 2c+1: partition p holds rows
            # 256c+p and 256c+128+p
            x3d = x.rearrange("(c two p) k -> p c two k", two=2, p=P)

            xstage = [None] * XC

            def sign_u16(dst, src):
                nc.vector.tensor_scalar(
                    dst, src, AND_MASK, OR_MASK,
                    mybir.AluOpType.bitwise_and, mybir.AluOpType.bitwise_or)

            def load_w(j):
                wr = ws_pool.tile([P, 2, N_SH], FP8, tag="wr")
                nc.gpsimd.dma_start(out=wr[:], in_=w4d[:, j])
                sign_u16(wsgn[:, j, :, :].bitcast(U16), wr[:].bitcast(U16))

            def load_x_raw(c):
                # raw staging for PE-transposed chunks (sign happens at the
                # psum eviction)
                xs = xs_pool.tile([P, 2, K], FP8, tag="xs")
                nc.gpsimd.dma_start(out=xs[:], in_=x3d[:, c])
                xstage[c] = xs

            def load_x_raw_half(c, half):
                if half == 0:
                    xstage[c] = xs_pool.tile([P, 2, K], FP8, tag="xs", name="xsh")
                nc.gpsimd.dma_start(
                    out=xstage[c][:, half, :], in_=x3d[:, c, half, :])

            def sign_x(c):
                # in-place sign of a raw-staged chunk (DVE); emitted in need
                # order so it can never block a due psum eviction
                sign_u16(xstage[c][:].bitcast(U16), xstage[c][:].bitcast(U16))

            def pe_transpose_half(c, half):
                # One m-block (16 u16-blocks) of chunk c through the PE in
                # two TGRP groups; sign is fused into the DVE eviction.
                # The PE transpose runs on BF16 *views* of the u16 pair
                # data: transpose mode is pure routing and bit-preserving
                # (verified on HW for all 65536 patterns), and bf16 is a
                # compiler-accepted PE dtype while u16 is not.  The psum
                # tiles are F32-shaped so the same pool doubles as warmup
                # accumulator space.
                xu = xstage[c][:].bitcast(mybir.dt.bfloat16)   # [P, 2, 2048]
                mbi = 2 * c + half
                for g in range(2):
                    pt = psumT_pool.tile([P, 512], F32, tag="pt", name="pt")
                    ptb = pt[:].bitcast(mybir.dt.bfloat16)     # [P, 1024]
                    for i in range(TGRP):
                        b = TGRP * g + i
                        nc.tensor.transpose(
                            ptb[:, i * P:(i + 1) * P],
                            xu[:, half, b * P:(b + 1) * P],
                            ident[:].bitcast(mybir.dt.bfloat16))
                    sign_u16(xT[:, mbi, TGRP * g:TGRP * (g + 1), :],
                             pt[:].bitcast(U16).rearrange(
                                 "p (a b) -> p a b", a=TGRP))

            def transpose_x(c):
                nc.sync.dma_start(
                    out=xT[:, 2 * c:2 * c + 2, :, :],
                    in_=xstage[c][:].bitcast(U16), transpose=True)

            def mm(po, mb, j, nb, start, stop):
                nc.tensor.matmul(
                    po[:], xT[:, mb, j, :].bitcast(FP8),
                    wsgn[:, j, :, nb * 512:(nb + 1) * 512],
                    start=start, stop=stop, perf_mode=DRSW)

            # ---- prologue loads: xc0, then the unbroken w stream, xc1 ----
            # Input DMAs after the w stream are pinned (tile_wait_until) to a
            # hand-planned timeline so the scheduler's enforced DMA order
            # matches what is actually achievable at runtime.
            load_w(0)
            load_w(1)
            load_x_raw(0)
            # identity built after the first loads so its gpsimd memset does
            # not delay the w stream's first SWDGE preps
            make_identity(nc, ident)
            for j in range(2, JB):
                load_w(j)
            with tc.tile_wait_until(PINS['xc1']):
                load_x_raw(1)

            # c0 PE transposes: gated only on the xc0 load; run pre-warmup
            pe_transpose_half(0, 0)
            pe_transpose_half(0, 1)

            # ---- warmup: mb0+mb1 j-interleaved, 6 matmul banks:
            # mb0 all 4 chunks, mb1 chunks 0-1; mb1 chunks 2-3 follow
            # nb-major right after.
            po6 = [psum_pool.tile([P, 512], F32, tag="po", name=f"po{i}")
                   for i in range(6)]
            for j in range(JB):
                for nb in range(NB):
                    mm(po6[nb], 0, j, nb, start=(j == 0), stop=(j == JB - 1))
                for nb in range(2):
                    mm(po6[4 + nb], 1, j, nb,
                       start=(j == 0), stop=(j == JB - 1))

            ob0 = ob_pool.tile([P, N_SH], I16, tag="ob")
            for nb in range(NB):
                nc.vector.tensor_copy(
                    out=ob0[:, nb * 512:(nb + 1) * 512], in_=po6[nb][:])
            nc.scalar.dma_start(out=out[0:P, :], in_=ob0[:])

            ob1 = ob_pool.tile([P, N_SH], I16, tag="ob")
            for nb in range(2):
                nc.vector.tensor_copy(
                    out=ob1[:, nb * 512:(nb + 1) * 512], in_=po6[4 + nb][:])
            # mb1 chunks 2-3 (nb-major) while c1 transposes interleave
            for nb in range(2, NB):
                po = psum_pool.tile([P, 512], F32, tag="po", name="po")
                for j in range(JB):
                    mm(po, 1, j, nb, start=(j == 0), stop=(j == JB - 1))
                if nb == 2:
                    pe_transpose_half(1, 0)
                nc.vector.tensor_copy(
                    out=ob1[:, nb * 512:(nb + 1) * 512], in_=po[:])
            nc.scalar.dma_start(out=out[P:2 * P, :], in_=ob1[:])

            # XBAR-transposed chunks: loads stream right after w on Pool;
            # signs/transposes are emitted in need order below
            for c, ms in ((2, PINS['c2']), (3, PINS['c3']), (4, PINS['c4']),
                          (5, PINS['c5']), (6, PINS['c6'])):
                with tc.tile_wait_until(ms):
                    load_x_raw(c)
            sign_x(2)
            with tc.tile_wait_until(PINS['T2']):
                transpose_x(2)
            sign_x(3)
            with tc.tile_wait_until(PINS['T3']):
                transpose_x(3)

            # ---- steady: mb2..15, nb-major, incremental bank recycling ----
            T_PINS = {4: PINS['T4'], 5: PINS['T5'], 6: PINS['T6'],
                      7: PINS['T7']}
            for mb in range(2, MB):
                if mb == 2:
                    pe_transpose_half(1, 1)
                if 2 <= mb <= 4:
                    sign_x(mb + 2)
                    with tc.tile_wait_until(T_PINS[mb + 2]):
                        transpose_x(mb + 2)
                elif mb == 5:
                    with tc.tile_wait_until(PINS['c7']):
                        load_x_raw(7)
                    sign_x(7)
                    with tc.tile_wait_until(T_PINS[7]):
                        transpose_x(7)
                last = (mb == MB - 1)
                ob = ob_pool.tile([P, N_SH], I16, tag="ob")
                for nb in range(NB):
                    po = psum_pool.tile([P, 512], F32, tag="po", name="po")
                    for j in range(JB):
                        mm(po, mb, j, nb, start=(j == 0), stop=(j == JB - 1))
                    nsl = slice(nb * 512, (nb + 1) * 512)
                    if not last:
                        nc.vector.tensor_copy(out=ob[:, nsl], in_=po[:])
                    else:
                        nc.vector.tensor_copy(out=ob[:, nsl], in_=po[:])
                        nc.sync.dma_start(
                            out=out[mb * P:(mb + 1) * P, nsl], in_=ob[:, nsl])
                if not last:
                    nc.scalar.dma_start(
                        out=out[mb * P:(mb + 1) * P, :], in_=ob[:])

    nc.compile()
    return nc


def get_nc():
    global _NC_CACHE
    if _NC_CACHE is None:
        _NC_CACHE = build_nc()
    return _NC_CACHE


def kernel(x: np.ndarray, w: np.ndarray) -> np.ndarray:
    x = np.asarray(x, dtype=np.float32)
    w = np.asarray(w, dtype=np.float32)
    assert x.shape == (M_FULL, K) and w.shape == (K, N_FULL)

    nc = get_nc()
    in_maps = []
    for c in range(N_CORES):
        mi, ni = divmod(c, RN)
        # SwInterleave reads stationary columns in reverse order: pre-reverse
        # x rows within each 128-row block so output rows land in order.
        xs = x[mi * M_SH:(mi + 1) * M_SH, :]
        xs = xs.reshape(MB, P, K)[:, ::-1, :].reshape(M_SH, K)
        in_maps.append({
            "x": np.ascontiguousarray(xs),
            "w": np.ascontiguousarray(w[:, ni * N_SH:(ni + 1) * N_SH]),
        })
    res = run_bass_kernel_spmd(nc, in_maps, list(range(N_CORES)))

    out = np.empty((M_FULL, N_FULL), dtype=np.float32)
    for c in range(N_CORES):
        mi, ni = divmod(c, RN)
        out[mi * M_SH:(mi + 1) * M_SH, ni * N_SH:(ni + 1) * N_SH] = \
            res.results[c]["out"].astype(np.float32)
    return out


# revision 6
# speedup vs baseline: 1.2642x; 1.0005x over previous
"""BinaryDense kernel for Trainium2: out = sign(x) @ sign(w).

Full shapes: x [8192, 4096] f32, w [4096, 4096] f32 -> out [8192, 4096] f32.
Sharding over 8 NeuronCores: x rows split 4 ways, w columns split 2 ways;
each core computes a [2048, 2048] block.  No collectives.

Core ideas (cost-model-driven):
  - fp8e5 cast-loads (SWDGE): IEEE casts preserve the sign BIT (even on
    underflow to +-0) and only the sign bit matters -> input DMA halves.
  - Bitwise sign on uint16 views: (r & 0x8080) | 0x3C3C == +-1.0 fp8e5 in
    both packed bytes.  Single DVE op per 2 elements, exact.
  - Matmuls: fp8 DoubleRowSwInterleave (0.5 cycles/row).  The k-pair-packed
    transposed x IS the interleaved stationary operand; w cast-loads land
    directly in the plane-separated moving layout [p, j, t, n].  The mode
    reads stationary columns in reverse order, so the host pre-reverses x
    rows within each 128-row block.
  - PSUM f32 accumulation is exact (+-1 products); int16 out; host widens.

Schedule: the finish time is ~(w-stream end + 97us), so the w stream runs
UNBROKEN on the Pool/SWDGE queue right after the first x chunk.  The first
two x chunks are transposed on the PE (raw fp8 pairs as u16; the sign is
fused into the DVE psum->SBUF eviction), which costs PE time only where PE
is delivery-paced anyway and keeps cross-queue DMA hops out of the w
stream.  Remaining chunks use XBAR DMA-transposes after the stream, where
the DMA device has slack.  Steady-state matmuls run nb-major so psum banks
recycle incrementally (6 matmul banks + 2 transpose banks).

Queue map: Pool=cast loads | DVE=signs+evictions | SP=XBAR transposes |
ACT=output DMA issue | PE=matmuls + first-two-chunk transposes.
"""

import numpy as np

import concourse.mybir as mybir
import concourse.tile as tile
from concourse import bacc
from concourse.bass_utils import run_bass_kernel_spmd
from concourse.masks import make_identity

P = 128
N_CORES = 8
RM, RN = 4, 2
M_FULL, K, N_FULL = 8192, 4096, 4096
M_SH, N_SH = M_FULL // RM, N_FULL // RN   # 2048, 2048
MB = M_SH // P           # 16 m-blocks
JB = K // 256            # 16 k-groups (DoubleRow: 2 planes x 128)
NB = N_SH // 512         # 4 psum-width chunks
XC = MB // 2             # 8 x-chunks of 2 m-blocks
TGRP = 8                 # u16 128-blocks per PE-transpose psum group

F32 = mybir.dt.float32
FP8 = mybir.dt.float8e5
U16 = mybir.dt.uint16
I16 = mybir.dt.int16

AND_MASK = 0x8080
OR_MASK = 0x3C3C
DRSW = mybir.MatmulPerfMode.DoubleRowSwInterleave

_NC_CACHE = None

# DMA schedule pins in "ms" for tc.tile_wait_until (1e6 ns units)
PINS = {
    "xc1": 0.0264, "c2": 0.0294, "c3": 0.0324, "c4": 0.0354,
    "c5": 0.0423, "c6": 0.0488, "c7": 0.0517,
    "T2": 0.0387, "T3": 0.0452, "T4": 0.0546, "T5": 0.0614,
    "T6": 0.0674, "T7": 0.0720,
}


def build_nc():
    nc = bacc.Bacc("TRN2", target_bir_lowering=False, debug=False,
                   num_devices=N_CORES)
    x = nc.dram_tensor("x", [M_SH, K], F32, kind="ExternalInput").ap()
    w = nc.dram_tensor("w", [K, N_SH], F32, kind="ExternalInput").ap()
    out = nc.dram_tensor("out", [M_SH, N_SH], I16, kind="ExternalOutput").ap()

    with tile.TileContext(nc) as tc:
        with (
            tc.tile_pool(name="const", bufs=1) as const_pool,
            tc.tile_pool(name="xT", bufs=1) as xT_pool,
            tc.tile_pool(name="wbin", bufs=1) as w_pool,
            tc.tile_pool(name="xs", bufs=5) as xs_pool,
            tc.tile_pool(name="ws", bufs=5) as ws_pool,
            tc.tile_pool(name="obuf", bufs=4) as ob_pool,
            tc.tile_pool(name="psum", bufs=6, space="PSUM") as psum_pool,
            tc.tile_pool(name="psumT", bufs=2, space="PSUM") as psumT_pool,
        ):
            ident = const_pool.tile([P, P], mybir.dt.int16)

            # xT u16[p, mb, j, m] = fp8 pair (k=256j+2p, +1) of row m
            xT = xT_pool.tile([P, MB, JB, P], U16)
            # wsgn[p, j, t, n] = sign(w[256j+2p+t, n])
            wsgn = w_pool.tile([P, JB, 2, N_SH], FP8)
            w4d = w.rearrange("(j p t) n -> p j t n", p=P, t=2)
            # x chunk c covers m-blocks 2c, 2c+1: partition p holds rows
            # 256c+p and 256c+128+p
            x3d = x.rearrange("(c two p) k -> p c two k", two=2, p=P)

            xstage = [None] * XC

            def sign_u16(dst, src):
                nc.vector.tensor_scalar(
                    dst, src, AND_MASK, OR_MASK,
                    mybir.AluOpType.bitwise_and, mybir.AluOpType.bitwise_or)

            def load_w(j):
                wr = ws_pool.tile([P, 2, N_SH], FP8, tag="wr")
                nc.gpsimd.dma_start(out=wr[:], in_=w4d[:, j])
                sign_u16(wsgn[:, j, :, :].bitcast(U16), wr[:].bitcast(U16))

            def load_x_raw(c):
                # raw staging for PE-transposed chunks (sign happens at the
                # psum eviction)
                xs = xs_pool.tile([P, 2, K], FP8, tag="xs")
                nc.gpsimd.dma_start(out=xs[:], in_=x3d[:, c])
                xstage[c] = xs

            def load_x_raw_half(c, half):
                if half == 0:
                    xstage[c] = xs_pool.tile([P, 2, K], FP8, tag="xs", name="xsh")
                nc.gpsimd.dma_start(
                    out=xstage[c][:, half, :], in_=x3d[:, c, half, :])

            def sign_x(c):
                # in-place sign of a raw-staged chunk (DVE); emitted in need
                # order so it can never block a due psum eviction
                sign_u16(xstage[c][:].bitcast(U16), xstage[c][:].bitcast(U16))

            def pe_transpose_half(c, half):
                # One m-block (16 u16-blocks) of chunk c through the PE in
                # two TGRP groups; sign is fused into the DVE eviction.
                # The PE transpose runs on BF16 *views* of the u16 pair
                # data: transpose mode is pure routing and bit-preserving
                # (verified on HW for all 65536 patterns), and bf16 is a
                # compiler-accepted PE dtype while u16 is not.  The psum
                # tiles are F32-shaped so the same pool doubles as warmup
                # accumulator space.
                xu = xstage[c][:].bitcast(mybir.dt.bfloat16)   # [P, 2, 2048]
                mbi = 2 * c + half
                for g in range(2):
                    pt = psumT_pool.tile([P, 512], F32, tag="pt", name="pt")
                    ptb = pt[:].bitcast(mybir.dt.bfloat16)     # [P, 1024]
                    for i in range(TGRP):
                        b = TGRP * g + i
                        nc.tensor.transpose(
                            ptb[:, i * P:(i + 1) * P],
                            xu[:, half, b * P:(b + 1) * P],
                            ident[:].bitcast(mybir.dt.bfloat16))
                    sign_u16(xT[:, mbi, TGRP * g:TGRP * (g + 1), :],
                             pt[:].bitcast(U16).rearrange(
                                 "p (a b) -> p a b", a=TGRP))

            def transpose_x(c):
                nc.sync.dma_start(
                    out=xT[:, 2 * c:2 * c + 2, :, :],
                    in_=xstage[c][:].bitcast(U16), transpose=True)

            def mm(po, mb, j, nb, start, stop):
                nc.tensor.matmul(
                    po[:], xT[:, mb, j, :].bitcast(FP8),
                    wsgn[:, j, :, nb * 512:(nb + 1) * 512],
                    start=start, stop=stop, perf_mode=DRSW)

            def mm2(po, mb, j, off, wd, start, stop):
                nc.tensor.matmul(
                    po[:, 0:wd], xT[:, mb, j, :].bitcast(FP8),
                    wsgn[:, j, :, off:off + wd],
                    start=start, stop=stop, perf_mode=DRSW)

            # ---- prologue loads: xc0, then the unbroken w stream, xc1 ----
            # Input DMAs after the w stream are pinned (tile_wait_until) to a
            # hand-planned timeline so the scheduler's enforced DMA order
            # matches what is actually achievable at runtime.
            load_w(0)
            load_w(1)
            load_x_raw(0)
            # identity built after the first loads so its gpsimd memset does
            # not delay the w stream's first SWDGE preps
            make_identity(nc, ident)
            for j in range(2, JB):
                load_w(j)
            with tc.tile_wait_until(PINS['xc1']):
                load_x_raw(1)

            # c0 PE transposes: gated only on the xc0 load; run pre-warmup
            pe_transpose_half(0, 0)
            pe_transpose_half(0, 1)

            # ---- warmup: mb0+mb1 j-interleaved, 6 matmul banks:
            # mb0 all 4 chunks, mb1 chunks 0-1; mb1 chunks 2-3 follow
            # nb-major right after.
            po6 = [psum_pool.tile([P, 512], F32, tag="po", name=f"po{i}")
                   for i in range(6)]
            for j in range(JB):
                for nb in range(NB):
                    mm(po6[nb], 0, j, nb, start=(j == 0), stop=(j == JB - 1))
                for nb in range(2):
                    mm(po6[4 + nb], 1, j, nb,
                       start=(j == 0), stop=(j == JB - 1))

            ob0 = ob_pool.tile([P, N_SH], I16, tag="ob")
            for nb in range(NB):
                nc.vector.tensor_copy(
                    out=ob0[:, nb * 512:(nb + 1) * 512], in_=po6[nb][:])
            nc.scalar.dma_start(out=out[0:P, :], in_=ob0[:])

            ob1 = ob_pool.tile([P, N_SH], I16, tag="ob")
            for nb in range(2):
                nc.vector.tensor_copy(
                    out=ob1[:, nb * 512:(nb + 1) * 512], in_=po6[4 + nb][:])
            # mb1 chunks 2-3 (nb-major) while c1 transposes interleave
            for nb in range(2, NB):
                po = psum_pool.tile([P, 512], F32, tag="po", name="po")
                for j in range(JB):
                    mm(po, 1, j, nb, start=(j == 0), stop=(j == JB - 1))
                if nb == 2:
                    pe_transpose_half(1, 0)
                nc.vector.tensor_copy(
                    out=ob1[:, nb * 512:(nb + 1) * 512], in_=po[:])
            nc.scalar.dma_start(out=out[P:2 * P, :], in_=ob1[:])

            # XBAR-transposed chunks: loads stream right after w on Pool;
            # signs/transposes are emitted in need order below
            for c, ms in ((2, PINS['c2']), (3, PINS['c3']), (4, PINS['c4']),
                          (5, PINS['c5']), (6, PINS['c6'])):
                with tc.tile_wait_until(ms):
                    load_x_raw(c)
            sign_x(2)
            with tc.tile_wait_until(PINS['T2']):
                transpose_x(2)
            sign_x(3)
            with tc.tile_wait_until(PINS['T3']):
                transpose_x(3)

            # ---- steady: mb2..15, nb-major, incremental bank recycling ----
            T_PINS = {4: PINS['T4'], 5: PINS['T5'], 6: PINS['T6'],
                      7: PINS['T7']}
            for mb in range(2, MB):
                if mb == 2:
                    pe_transpose_half(1, 1)
                if 2 <= mb <= 4:
                    sign_x(mb + 2)
                    with tc.tile_wait_until(T_PINS[mb + 2]):
                        transpose_x(mb + 2)
                elif mb == 5:
                    with tc.tile_wait_until(PINS['c7']):
                        load_x_raw(7)
                    sign_x(7)
                    with tc.tile_wait_until(T_PINS[7]):
                        transpose_x(7)
                last = (mb == MB - 1)
                ob = ob_pool.tile([P, N_SH], I16, tag="ob")
                # the last m-block tapers its final chunks (384 then 128 wide)
                # so the exposed end-of-program evict+DMA chain is short
                widths = [512, 512, 512, 384, 128] if last else [512] * NB
                off = 0
                for wd in widths:
                    po = psum_pool.tile([P, 512], F32, tag="po", name="po")
                    for j in range(JB):
                        mm2(po, mb, j, off, wd,
                            start=(j == 0), stop=(j == JB - 1))
                    nsl = slice(off, off + wd)
                    nc.vector.tensor_copy(out=ob[:, nsl], in_=po[:, 0:wd])
                    if last:
                        nc.sync.dma_start(
                            out=out[mb * P:(mb + 1) * P, nsl], in_=ob[:, nsl])
                    off += wd
                if not last:
                    nc.scalar.dma_start(
                        out=out[mb * P:(mb + 1) * P, :], in_=ob[:])

    nc.compile()
    return nc


def get_nc():
    global _NC_CACHE
    if _NC_CACHE is None:
        _NC_CACHE = build_nc()
    return _NC_CACHE


def kernel(x: np.ndarray, w: np.ndarray) -> np.ndarray:
    x = np.asarray(x, dtype=np.float32)
    w = np.asarray(w, dtype=np.float32)
    assert x.shape == (M_FULL, K) and w.shape == (K, N_FULL)

    nc = get_nc()
    in_maps = []
    for c in range(N_CORES):
        mi, ni = divmod(c, RN)
        # SwInterleave reads stationary columns in reverse order: pre-reverse
        # x rows within each 128-row block so output rows land in order.
        xs = x[mi * M_SH:(mi + 1) * M_SH, :]
        xs = xs.reshape(MB, P, K)[:, ::-1, :].reshape(M_SH, K)
        in_maps.append({
            "x": np.ascontiguousarray(xs),
            "w": np.ascontiguousarray(w[:, ni * N_SH:(ni + 1) * N_SH]),
        })
    res = run_bass_kernel_spmd(nc, in_maps, list(range(N_CORES)))

    out = np.empty((M_FULL, N_FULL), dtype=np.float32)
    for c in range(N_CORES):
        mi, ni = divmod(c, RN)
        out[mi * M_SH:(mi + 1) * M_SH, ni * N_SH:(ni + 1) * N_SH] = \
            res.results[c]["out"].astype(np.float32)
    return out
